# revision 15
# baseline (speedup 1.0000x reference)
"""MultiHeadAttention Trainium2 kernel: 8-core SPMD (batch x head-group sharding).

Problem: B=2, S=2048, E=1024, H=16, D=64. nn.MultiheadAttention forward:
  Q = q @ Wq.T + bq; K,V likewise; softmax(Q Kh^T / sqrt(E)) V per head;
  out = concat_heads @ Wo.T + bo.

Sharding: core c -> batch b = c//4, head group g = c%4 (heads 4g..4g+3,
feature slice 256g..256g+256). Each core computes a partial output
projection [S, E] for its batch; host sums the 4 partials per batch and
adds bo (cheaper than a device all-reduce at this size).

All device matmuls run in float32r (TF32-like, 1 cyc/row at N>=256).
Layout trick: host passes x transposed (feature-major) so projections and
attention need no on-device transposes. Attention computes S^T = K^T.T @ Q^T
per head so softmax sums fold into the A@V matmul via ones-columns
appended to V (PSUM rows 64:128 = broadcast softmax denominators).
"""
import numpy as np

_CACHE = {}

B, S, E, H, D = 2, 2048, 1024, 16, 64
N_CORES = 8
HEADS_PER_CORE = 4  # 256-wide feature slice per core
JS = HEADS_PER_CORE * D  # 256
SCALE = 1.0 / np.sqrt(np.float32(E))  # note: embed_dim scaling, not head_dim


def _build(n_iter=1):
    import concourse.bacc as bacc
    import concourse.mybir as mybir
    import concourse.tile as tile
    from concourse import bass

    f32 = mybir.dt.float32
    f32r = mybir.dt.float32r
    AF = mybir.ActivationFunctionType

    nc = bacc.Bacc("TRN2", target_bir_lowering=False, debug=False,
                   num_devices=N_CORES)

    xqT = nc.dram_tensor("xqT", [E, S], f32, kind="ExternalInput").ap()
    xkT = nc.dram_tensor("xkT", [E, S], f32, kind="ExternalInput").ap()
    xvT = nc.dram_tensor("xvT", [E, S], f32, kind="ExternalInput").ap()
    wqT = nc.dram_tensor("wqT", [E, JS], f32, kind="ExternalInput").ap()
    wkT = nc.dram_tensor("wkT", [E, JS], f32, kind="ExternalInput").ap()
    wvT = nc.dram_tensor("wvT", [E, JS], f32, kind="ExternalInput").ap()
    woT = nc.dram_tensor("woT", [JS, E], f32, kind="ExternalInput").ap()
    bq = nc.dram_tensor("bq", [1, JS], f32, kind="ExternalInput").ap()
    bk = nc.dram_tensor("bk", [1, JS], f32, kind="ExternalInput").ap()
    bv = nc.dram_tensor("bv", [1, JS], f32, kind="ExternalInput").ap()
    yT = nc.dram_tensor("yT", [E, S], f32, kind="ExternalOutput").ap()

    FC = E // 128        # 8 feature chunks
    TC = 8               # t-chunks for projection stage
    TCS = S // TC        # 256 tokens per chunk
    NTK = S // 128       # 16 key tiles
    NTQ = 2              # tq chunks of 1024 in attention
    TQS = S // NTQ       # 1024

    with tile.TileContext(nc) as tc:
        from contextlib import ExitStack
        ctx = ExitStack()
        with ctx:
            wpool = ctx.enter_context(tc.tile_pool(name="wpool", bufs=1))
            xpool = ctx.enter_context(tc.tile_pool(name="xpool", bufs=2))
            spool = ctx.enter_context(tc.tile_pool(name="spool", bufs=1))
            ppool = ctx.enter_context(tc.tile_pool(name="ppool", bufs=2))
            rpool = ctx.enter_context(tc.tile_pool(name="rpool", bufs=2))
            ypool = ctx.enter_context(tc.tile_pool(name="ypool", bufs=3))
            psA = ctx.enter_context(tc.tile_pool(name="psA", bufs=2, space="PSUM"))
            psS = ctx.enter_context(tc.tile_pool(name="psS", bufs=2, space="PSUM"))
            psO = ctx.enter_context(tc.tile_pool(name="psO", bufs=1, space="PSUM"))

            if n_iter > 1:
                _loop = tc.For_i(0, n_iter, 1)
                _loop.__enter__()

            # ---- resident weights / constants (f32r via SWDGE cast DMA) ----
            wq_s = wpool.tile([128, FC, JS], f32r, tag="wq")
            wk_s = wpool.tile([128, FC, JS], f32r, tag="wk")
            wv_s = wpool.tile([128, FC, JS], f32r, tag="wv")
            nc.gpsimd.dma_start(out=wq_s, in_=wqT.rearrange("(c k) j -> k c j", c=FC))
            nc.gpsimd.dma_start(out=wk_s, in_=wkT.rearrange("(c k) j -> k c j", c=FC))
            nc.gpsimd.dma_start(out=wv_s, in_=wvT.rearrange("(c k) j -> k c j", c=FC))
            wo_s = wpool.tile([128, 2, E], f32r, tag="wo")
            nc.gpsimd.dma_start(out=wo_s, in_=woT.rearrange("(c j) e -> j c e", c=2))
            bq_s = wpool.tile([1, JS], f32r, tag="bq")
            bk_s = wpool.tile([1, JS], f32r, tag="bk")
            bv_s = wpool.tile([1, JS], f32r, tag="bv")
            nc.gpsimd.dma_start(out=bq_s, in_=bq)
            nc.gpsimd.dma_start(out=bk_s, in_=bk)
            nc.gpsimd.dma_start(out=bv_s, in_=bv)
            ones_sc = wpool.tile([128, 1024], f32, tag="ones_sc")
            nc.vector.memset(ones_sc, 1.0)
            ones = wpool.tile([1, TCS], f32r, tag="ones")
            nc.vector.tensor_copy(ones, ones_sc[0:1, :TCS])

            # ---- stage A outputs (resident, f32r) ----
            qt = spool.tile([128, 2, S], f32r, tag="qt")    # Q^T  [256, S]
            kt = spool.tile([128, 2, S], f32r, tag="kt")    # K^T  [256, S]
            # V augmented, per head h a contiguous block of 128 cols:
            # cols 128h..128h+63 = V head h, cols 128h+64..128h+127 = ones
            # (PSUM rows 64:128 of the A@V matmul then hold softmax sums)
            vaug = spool.tile([128, NTK, 512], f32r, tag="vaug")
            for h in range(HEADS_PER_CORE):
                nc.vector.tensor_copy(
                    vaug[:, :, 128 * h + 64:128 * (h + 1)],
                    ones_sc.rearrange("p (n c) -> p n c", c=64))
            # O^T (normalized attention out, head-major)  [256, S]
            ot = spool.tile([128, 2, S], f32r, tag="ot")

            xq_r = xqT.rearrange("(c k) t -> k c t", c=FC)
            xk_r = xkT.rearrange("(c k) t -> k c t", c=FC)
            xv_r = xvT.rearrange("(c k) t -> k c t", c=FC)

            # ---- stage A: projections ----
            for ti in range(TC):
                t0 = ti * TCS
                xq_c = xpool.tile([128, FC, TCS], f32r, tag="xq")
                xk_c = xpool.tile([128, FC, TCS], f32r, tag="xk")
                xv_c = xpool.tile([128, FC, TCS], f32r, tag="xv")
                nc.gpsimd.dma_start(out=xq_c, in_=xq_r[:, :, t0:t0 + TCS])
                nc.gpsimd.dma_start(out=xk_c, in_=xk_r[:, :, t0:t0 + TCS])
                nc.gpsimd.dma_start(out=xv_c, in_=xv_r[:, :, t0:t0 + TCS])

                for w_s, b_s, x_c, dest in ((wq_s, bq_s, xq_c, qt),
                                            (wk_s, bk_s, xk_c, kt)):
                    for j in range(2):
                        ps = psA.tile([128, 512], f32, tag="mm")
                        pm = ps[:, :TCS]
                        for f in range(FC):
                            nc.tensor.matmul(pm, w_s[:, f, 128 * j:128 * (j + 1)],
                                             x_c[:, f], start=(f == 0), stop=False)
                        nc.tensor.matmul(pm, b_s[:, 128 * j:128 * (j + 1)], ones,
                                         start=False, stop=True)
                        nc.vector.tensor_copy(dest[:, j, t0:t0 + TCS], pm)

                for tt in range(TCS // 128):
                    tidx = (t0 + tt * 128) // 128
                    ps = psA.tile([128, 512], f32, tag="mm")
                    pm = ps[:, :JS]
                    for f in range(FC):
                        nc.tensor.matmul(pm, xv_c[:, f, tt * 128:(tt + 1) * 128],
                                         wv_s[:, f], start=(f == 0), stop=False)
                    nc.tensor.matmul(pm, ones[:, :128], bv_s, start=False, stop=True)
                    nc.vector.tensor_copy(
                        vaug.rearrange("p n (h c) -> p n h c", c=128)[:, tidx, :, :64],
                        pm.rearrange("p (h c) -> p h c", c=64))

            # ---- stage B: attention per head ----
            for h in range(HEADS_PER_CORE):
                jt, jp = h // 2, 64 * (h % 2)
                for tq in range(NTQ):
                    q0 = tq * TQS
                    po = psO.tile([128, TQS], f32, tag="av")
                    for tk in range(NTK):
                        pst = psS.tile([128, TQS], f32, tag="st")
                        for half in range(2):
                            nc.tensor.matmul(
                                pst[:, half * 512:(half + 1) * 512],
                                kt[jp:jp + 64, jt, tk * 128:(tk + 1) * 128],
                                qt[jp:jp + 64, jt, q0 + half * 512:q0 + (half + 1) * 512],
                                start=True, stop=True)
                        pt = ppool.tile([128, TQS], f32r, tag="pt")
                        nc.scalar.activation(pt, pst, AF.Exp, scale=float(SCALE))
                        lhs_av = vaug[:, tk, 128 * h:128 * (h + 1)]
                        for half in range(2):
                            nc.tensor.matmul(
                                po[:, half * 512:(half + 1) * 512],
                                lhs_av,
                                pt[:, half * 512:(half + 1) * 512],
                                start=(tk == 0), stop=(tk == NTK - 1))
                    # rows 64:128 of po are softmax sums (broadcast x64)
                    rt = rpool.tile([64, TQS], f32, tag="rt")
                    nc.vector.reciprocal(rt, po[64:128, :])
                    nc.vector.tensor_tensor(
                        ot[jp:jp + 64, jt, q0:q0 + TQS], po[0:64, :], rt,
                        op=mybir.AluOpType.mult)

            # ---- stage C: output projection (partial) ----
            for e in range(8):
                for t4 in range(4):
                    ps = psA.tile([128, 512], f32, tag="mm")
                    for j in range(2):
                        nc.tensor.matmul(ps, wo_s[:, j, e * 128:(e + 1) * 128],
                                         ot[:, j, t4 * 512:(t4 + 1) * 512],
                                         start=(j == 0), stop=(j == 1))
                    yst = ypool.tile([128, 512], f32, tag="yst")
                    nc.vector.tensor_copy(yst, ps)
                    nc.sync.dma_start(out=yT[e * 128:(e + 1) * 128,
                                             t4 * 512:(t4 + 1) * 512], in_=yst)

            if n_iter > 1:
                _loop.__exit__(None, None, None)

    nc.compile()
    return nc


def _get_runner():
    if "runner" in _CACHE:
        return _CACHE["runner"]
    import time
    import jax
    from jax.sharding import Mesh, PartitionSpec
    from jax.experimental.shard_map import shard_map
    import concourse.mybir as mybir
    from concourse.bass2jax import (_bass_exec_p, partition_id_tensor,
                                    install_neuronx_cc_hook)

    nc = _build()
    install_neuronx_cc_hook()
    partition_name = nc.partition_id_tensor.name if nc.partition_id_tensor else None
    in_names, out_names, out_avals, zero_outs = [], [], [], []
    for alloc in nc.m.functions[0].allocations:
        if not isinstance(alloc, mybir.MemoryLocationSet):
            continue
        name = alloc.memorylocations[0].name
        if alloc.kind == "ExternalInput":
            if name != partition_name:
                in_names.append(name)
        elif alloc.kind == "ExternalOutput":
            out_names.append(name)
            np_dt = mybir.dt.np(alloc.dtype)
            out_avals.append(jax.core.ShapedArray(tuple(alloc.tensor_shape), np_dt))
            zero_outs.append(np.zeros(tuple(alloc.tensor_shape), np_dt))

    n_params = len(in_names)
    all_in_names = list(in_names) + list(out_names)
    if partition_name is not None:
        all_in_names.append(partition_name)

    def _body(*args):
        operands = list(args)
        if partition_name is not None:
            operands.append(partition_id_tensor())
        outs = _bass_exec_p.bind(
            *operands, out_avals=tuple(out_avals), in_names=tuple(all_in_names),
            out_names=tuple(out_names), lowering_input_output_aliases=(),
            sim_require_finite=True, sim_require_nnan=True, nc=nc)
        return tuple(outs)

    devices = jax.devices()[:N_CORES]
    mesh = Mesh(np.asarray(devices), ("core",))
    n_outs = len(out_names)
    fn = jax.jit(
        shard_map(_body, mesh=mesh,
                  in_specs=(PartitionSpec("core"),) * (n_params + n_outs),
                  out_specs=(PartitionSpec("core"),) * n_outs,
                  check_rep=False),
        keep_unused=True)

    runner = {"fn": fn, "in_names": in_names, "out_names": out_names,
              "out_avals": out_avals, "zero_outs": zero_outs, "jax": jax}
    _CACHE["nc"] = nc
    _CACHE["runner"] = runner
    return runner


def build_chained(n_chain):
    """Jitted fn running the kernel n_chain times back-to-back (serialized via
    a tiny data dependency through bq) — for slope-based device timing."""
    r = _get_runner()
    import jax
    from jax.sharding import Mesh, PartitionSpec
    from jax.experimental.shard_map import shard_map
    from concourse.bass2jax import _bass_exec_p, partition_id_tensor

    nc = _CACHE["nc"]
    partition_name = nc.partition_id_tensor.name if nc.partition_id_tensor else None
    in_names = r["in_names"]
    out_names = r["out_names"]
    out_avals = r["out_avals"]
    n_params = len(in_names)
    all_in_names = list(in_names) + list(out_names)
    if partition_name is not None:
        all_in_names.append(partition_name)
    bq_idx = in_names.index("bq")
    yt_idx = out_names.index("yT")

    def _once(args):
        operands = list(args)
        if partition_name is not None:
            operands.append(partition_id_tensor())
        return _bass_exec_p.bind(
            *operands, out_avals=tuple(out_avals), in_names=tuple(all_in_names),
            out_names=tuple(out_names), lowering_input_output_aliases=(),
            sim_require_finite=True, sim_require_nnan=True, nc=nc)

    def _body(*args):
        args = list(args)
        outs = _once(args)
        for _ in range(n_chain - 1):
            # serialize: call i's output becomes call i+1's output buffer
            args[n_params + yt_idx] = outs[yt_idx]
            outs = _once(args)
        return tuple(outs)

    devices = jax.devices()[:N_CORES]
    mesh = Mesh(np.asarray(devices), ("core",))
    n_outs = len(out_names)
    return jax.jit(
        shard_map(_body, mesh=mesh,
                  in_specs=(PartitionSpec("core"),) * (n_params + n_outs),
                  out_specs=(PartitionSpec("core"),) * n_outs,
                  check_rep=False),
        keep_unused=True)


def _shard_inputs(query, key, value, Wq, bq, Wk, bk, Wv, bv, Wo, bo):
    """Build per-core input dict list."""
    q32 = np.asarray(query, dtype=np.float32)
    k32 = np.asarray(key, dtype=np.float32)
    v32 = np.asarray(value, dtype=np.float32)
    xT = [np.ascontiguousarray(a.transpose(0, 2, 1)) for a in (q32, k32, v32)]
    Wq, Wk, Wv, Wo = (np.asarray(a, np.float32) for a in (Wq, Wk, Wv, Wo))
    bqv, bkv, bvv = (np.asarray(a, np.float32).reshape(1, -1) for a in (bq, bk, bv))
    in_maps = []
    for c in range(N_CORES):
        b, g = divmod(c, HEADS_PER_CORE)
        j0 = g * JS
        in_maps.append({
            "xqT": xT[0][b], "xkT": xT[1][b], "xvT": xT[2][b],
            "wqT": np.ascontiguousarray(Wq[j0:j0 + JS].T),
            "wkT": np.ascontiguousarray(Wk[j0:j0 + JS].T),
            "wvT": np.ascontiguousarray(Wv[j0:j0 + JS].T),
            "woT": np.ascontiguousarray(Wo[:, j0:j0 + JS].T),
            "bq": bqv[:, j0:j0 + JS], "bk": bkv[:, j0:j0 + JS],
            "bv": bvv[:, j0:j0 + JS],
        })
    return in_maps


def kernel(query, key, value, Wq, bq, Wk, bk, Wv, bv, Wo, bo):
    r = _get_runner()
    jax = r["jax"]
    in_maps = _shard_inputs(query, key, value, Wq, bq, Wk, bk, Wv, bv, Wo, bo)
    concat_in = [np.concatenate([in_maps[c][nm] for c in range(N_CORES)], axis=0)
                 for nm in r["in_names"]]
    concat_zeros = [np.zeros((N_CORES * z.shape[0], *z.shape[1:]), z.dtype)
                    for z in r["zero_outs"]]
    outs = r["fn"](*[jax.device_put(a) for a in concat_in + concat_zeros])
    jax.block_until_ready(outs)
    i = r["out_names"].index("yT")
    yT_all = np.asarray(outs[i]).reshape(N_CORES, E, S)
    bo32 = np.asarray(bo, np.float32)
    out = np.empty((B, S, E), np.float32)
    for b in range(B):
        acc = yT_all[4 * b:4 * b + 4].sum(axis=0)  # [E, S]
        out[b] = acc.T + bo32
    return out


# revision 19
# speedup vs baseline: 1.0314x; 1.0314x over previous
"""MultiHeadAttention Trainium2 kernel: 8-core SPMD (batch x head-group sharding).

Problem: B=2, S=2048, E=1024, H=16, D=64. nn.MultiheadAttention forward:
  Q = q @ Wq.T + bq; K,V likewise; softmax(Q Kh^T / sqrt(E)) V per head;
  out = concat_heads @ Wo.T + bo.

Sharding: core c -> batch b = c//4, head group g = c%4 (heads 4g..4g+3,
feature slice 256g..256g+256). Each core computes a partial output
projection [S, E] for its batch; host sums the 4 partials per batch and
adds bo (cheaper than a device all-reduce at this size).

All device matmuls run in float32r (TF32-like, 1 cyc/row at N>=256).
Layout trick: host passes x transposed (feature-major) so projections and
attention need no on-device transposes. Attention computes S^T = K^T.T @ Q^T
per head so softmax sums fold into the A@V matmul via ones-columns
appended to V (PSUM rows 64:128 = broadcast softmax denominators).
"""
import numpy as np

_CACHE = {}

B, S, E, H, D = 2, 2048, 1024, 16, 64
N_CORES = 8
HEADS_PER_CORE = 4  # 256-wide feature slice per core
JS = HEADS_PER_CORE * D  # 256
SCALE = 1.0 / np.sqrt(np.float32(E))  # note: embed_dim scaling, not head_dim


def _build(n_iter=1):
    import concourse.bacc as bacc
    import concourse.mybir as mybir
    import concourse.tile as tile
    from concourse import bass

    f32 = mybir.dt.float32
    f32r = mybir.dt.float32r
    AF = mybir.ActivationFunctionType

    nc = bacc.Bacc("TRN2", target_bir_lowering=False, debug=False,
                   num_devices=N_CORES)

    xqT = nc.dram_tensor("xqT", [E, S], f32, kind="ExternalInput").ap()
    xkT = nc.dram_tensor("xkT", [E, S], f32, kind="ExternalInput").ap()
    xvT = nc.dram_tensor("xvT", [E, S], f32, kind="ExternalInput").ap()
    wqT = nc.dram_tensor("wqT", [E, JS], f32, kind="ExternalInput").ap()
    wkT = nc.dram_tensor("wkT", [E, JS], f32, kind="ExternalInput").ap()
    wvT = nc.dram_tensor("wvT", [E, JS], f32, kind="ExternalInput").ap()
    woT = nc.dram_tensor("woT", [JS, E], f32, kind="ExternalInput").ap()
    bq = nc.dram_tensor("bq", [1, JS], f32, kind="ExternalInput").ap()
    bk = nc.dram_tensor("bk", [1, JS], f32, kind="ExternalInput").ap()
    bv = nc.dram_tensor("bv", [1, JS], f32, kind="ExternalInput").ap()
    yT = nc.dram_tensor("yT", [E, S], f32, kind="ExternalOutput").ap()

    FC = E // 128        # 8 feature chunks
    TC = 8               # t-chunks for projection stage
    TCS = S // TC        # 256 tokens per chunk
    NTK = S // 128       # 16 key tiles
    NTQ = 2              # tq chunks of 1024 in attention
    TQS = S // NTQ       # 1024

    with tile.TileContext(nc) as tc:
        from contextlib import ExitStack
        ctx = ExitStack()
        with ctx:
            wpool = ctx.enter_context(tc.tile_pool(name="wpool", bufs=1))
            xpool = ctx.enter_context(tc.tile_pool(name="xpool", bufs=2))
            spool = ctx.enter_context(tc.tile_pool(name="spool", bufs=1))
            ppool = ctx.enter_context(tc.tile_pool(name="ppool", bufs=2))
            rpool = ctx.enter_context(tc.tile_pool(name="rpool", bufs=2))
            ypool = ctx.enter_context(tc.tile_pool(name="ypool", bufs=3))
            psA = ctx.enter_context(tc.tile_pool(name="psA", bufs=2, space="PSUM"))
            psS = ctx.enter_context(tc.tile_pool(name="psS", bufs=2, space="PSUM"))
            psO = ctx.enter_context(tc.tile_pool(name="psO", bufs=1, space="PSUM"))

            if n_iter > 1:
                _loop = tc.For_i(0, n_iter, 1)
                _loop.__enter__()

            # ---- resident weights / constants ----
            # HWDGE (nc.sync) f32 staging -> DVE cast to f32r: keeps the SWDGE
            # path free so the x stream starts immediately.
            wq_st = xpool.tile([128, FC, JS], f32, tag="xq", name="wq_st")
            wk_st = xpool.tile([128, FC, JS], f32, tag="xk", name="wk_st")
            wv_st = xpool.tile([128, FC, JS], f32, tag="xv", name="wv_st")
            wo_st = xpool.tile([128, 2, E], f32, tag="xq", name="wo_st")
            b_st = wpool.tile([1, 3, JS], f32, tag="b_st")
            nc.sync.dma_start(out=wq_st, in_=wqT.rearrange("(c k) j -> k c j", c=FC))
            nc.sync.dma_start(out=wk_st, in_=wkT.rearrange("(c k) j -> k c j", c=FC))
            nc.sync.dma_start(out=wv_st, in_=wvT.rearrange("(c k) j -> k c j", c=FC))
            nc.sync.dma_start(out=wo_st, in_=woT.rearrange("(c j) e -> j c e", c=2))
            nc.sync.dma_start(out=b_st[:, 0], in_=bq)
            nc.sync.dma_start(out=b_st[:, 1], in_=bk)
            nc.sync.dma_start(out=b_st[:, 2], in_=bv)
            wq_s = wpool.tile([128, FC, JS], f32r, tag="wq")
            wk_s = wpool.tile([128, FC, JS], f32r, tag="wk")
            wv_s = wpool.tile([128, FC, JS], f32r, tag="wv")
            wo_s = wpool.tile([128, 2, E], f32r, tag="wo")
            b_s = wpool.tile([1, 3, JS], f32r, tag="b_s")
            nc.vector.tensor_copy(wq_s, wq_st)
            nc.vector.tensor_copy(wk_s, wk_st)
            nc.vector.tensor_copy(wv_s, wv_st)
            nc.vector.tensor_copy(wo_s, wo_st)
            nc.vector.tensor_copy(b_s, b_st)
            bq_s, bk_s, bv_s = b_s[:, 0], b_s[:, 1], b_s[:, 2]
            ones_sc = wpool.tile([128, 1024], f32, tag="ones_sc")
            nc.vector.memset(ones_sc, 1.0)
            ones = wpool.tile([1, TCS], f32r, tag="ones")
            nc.vector.tensor_copy(ones, ones_sc[0:1, :TCS])

            # ---- stage A outputs (resident, f32r) ----
            qt = spool.tile([128, 2, S], f32r, tag="qt")    # Q^T  [256, S]
            kt = spool.tile([128, 2, S], f32r, tag="kt")    # K^T  [256, S]
            # V augmented, per head h a contiguous block of 128 cols:
            # cols 128h..128h+63 = V head h, cols 128h+64..128h+127 = ones
            # (PSUM rows 64:128 of the A@V matmul then hold softmax sums)
            vaug = spool.tile([128, NTK, 512], f32r, tag="vaug")
            for h in range(HEADS_PER_CORE):
                nc.vector.tensor_copy(
                    vaug[:, :, 128 * h + 64:128 * (h + 1)],
                    ones_sc.rearrange("p (n c) -> p n c", c=64))
            # O^T (normalized attention out, head-major)  [256, S]
            ot = spool.tile([128, 2, S], f32r, tag="ot")

            xq_r = xqT.rearrange("(c k) t -> k c t", c=FC)
            xk_r = xkT.rearrange("(c k) t -> k c t", c=FC)
            xv_r = xvT.rearrange("(c k) t -> k c t", c=FC)

            # ---- stage A: projections ----
            for ti in range(TC):
                t0 = ti * TCS
                xq_c = xpool.tile([128, FC, TCS], f32r, tag="xq")
                xk_c = xpool.tile([128, FC, TCS], f32r, tag="xk")
                xv_c = xpool.tile([128, FC, TCS], f32r, tag="xv")
                nc.gpsimd.dma_start(out=xq_c, in_=xq_r[:, :, t0:t0 + TCS])
                nc.gpsimd.dma_start(out=xk_c, in_=xk_r[:, :, t0:t0 + TCS])
                nc.gpsimd.dma_start(out=xv_c, in_=xv_r[:, :, t0:t0 + TCS])

                for w_s, b_s, x_c, dest in ((wq_s, bq_s, xq_c, qt),
                                            (wk_s, bk_s, xk_c, kt)):
                    for j in range(2):
                        ps = psA.tile([128, 512], f32, tag="mm")
                        pm = ps[:, :TCS]
                        for f in range(FC):
                            nc.tensor.matmul(pm, w_s[:, f, 128 * j:128 * (j + 1)],
                                             x_c[:, f], start=(f == 0), stop=False)
                        nc.tensor.matmul(pm, b_s[:, 128 * j:128 * (j + 1)], ones,
                                         start=False, stop=True)
                        nc.vector.tensor_copy(dest[:, j, t0:t0 + TCS], pm)

                for tt in range(TCS // 128):
                    tidx = (t0 + tt * 128) // 128
                    ps = psA.tile([128, 512], f32, tag="mm")
                    pm = ps[:, :JS]
                    for f in range(FC):
                        nc.tensor.matmul(pm, xv_c[:, f, tt * 128:(tt + 1) * 128],
                                         wv_s[:, f], start=(f == 0), stop=False)
                    nc.tensor.matmul(pm, ones[:, :128], bv_s, start=False, stop=True)
                    nc.vector.tensor_copy(
                        vaug.rearrange("p n (h c) -> p n h c", c=128)[:, tidx, :, :64],
                        pm.rearrange("p (h c) -> p h c", c=64))

            # ---- stages B+C interleaved over 512-token q blocks ----
            # Head pairs (2p, 2p+1) share kt/qt j-tile p at partition rows
            # 0:64 / 64:128 -> their S^T matmuls run concurrently on disjoint
            # PE row groups (K=64 each). One exp call covers both heads.
            for tq4 in range(4):
                q0 = tq4 * 512
                for pair in range(2):
                    po = psO.tile([128, 1024], f32, tag="av")
                    for tk in range(NTK):
                        pst = psS.tile([128, 1024], f32, tag="st")
                        for sub, jp in ((0, 0), (1, 64)):
                            nc.tensor.matmul(
                                pst[:, sub * 512:(sub + 1) * 512],
                                kt[jp:jp + 64, pair, tk * 128:(tk + 1) * 128],
                                qt[jp:jp + 64, pair, q0:q0 + 512],
                                start=True, stop=True)
                        pt = ppool.tile([128, 1024], f32r, tag="pt")
                        nc.scalar.activation(pt, pst, AF.Exp, scale=float(SCALE))
                        for sub in range(2):
                            h = 2 * pair + sub
                            nc.tensor.matmul(
                                po[:, sub * 512:(sub + 1) * 512],
                                vaug[:, tk, 128 * h:128 * (h + 1)],
                                pt[:, sub * 512:(sub + 1) * 512],
                                start=(tk == 0), stop=(tk == NTK - 1))
                    # rows 64:128 of po are softmax sums (broadcast x64)
                    for sub in range(2):
                        jp = 64 * sub
                        pos = po[:, sub * 512:(sub + 1) * 512]
                        rt = rpool.tile([64, 512], f32, tag="rt")
                        nc.vector.reciprocal(rt, pos[64:128, :])
                        nc.vector.tensor_tensor(
                            ot[jp:jp + 64, pair, q0:q0 + 512], pos[0:64, :], rt,
                            op=mybir.AluOpType.mult)
                # output projection for this q block (overlaps next block)
                for e in range(8):
                    ps = psA.tile([128, 512], f32, tag="mm")
                    for j in range(2):
                        nc.tensor.matmul(ps, wo_s[:, j, e * 128:(e + 1) * 128],
                                         ot[:, j, q0:q0 + 512],
                                         start=(j == 0), stop=(j == 1))
                    yst = ypool.tile([128, 512], f32, tag="yst")
                    nc.vector.tensor_copy(yst, ps)
                    nc.sync.dma_start(out=yT[e * 128:(e + 1) * 128,
                                             q0:q0 + 512], in_=yst)

            if n_iter > 1:
                _loop.__exit__(None, None, None)

    nc.compile()
    return nc


def _get_runner():
    if "runner" in _CACHE:
        return _CACHE["runner"]
    import time
    import jax
    from jax.sharding import Mesh, PartitionSpec
    from jax.experimental.shard_map import shard_map
    import concourse.mybir as mybir
    from concourse.bass2jax import (_bass_exec_p, partition_id_tensor,
                                    install_neuronx_cc_hook)

    nc = _build()
    install_neuronx_cc_hook()
    partition_name = nc.partition_id_tensor.name if nc.partition_id_tensor else None
    in_names, out_names, out_avals, zero_outs = [], [], [], []
    for alloc in nc.m.functions[0].allocations:
        if not isinstance(alloc, mybir.MemoryLocationSet):
            continue
        name = alloc.memorylocations[0].name
        if alloc.kind == "ExternalInput":
            if name != partition_name:
                in_names.append(name)
        elif alloc.kind == "ExternalOutput":
            out_names.append(name)
            np_dt = mybir.dt.np(alloc.dtype)
            out_avals.append(jax.core.ShapedArray(tuple(alloc.tensor_shape), np_dt))
            zero_outs.append(np.zeros(tuple(alloc.tensor_shape), np_dt))

    n_params = len(in_names)
    all_in_names = list(in_names) + list(out_names)
    if partition_name is not None:
        all_in_names.append(partition_name)

    def _body(*args):
        operands = list(args)
        if partition_name is not None:
            operands.append(partition_id_tensor())
        outs = _bass_exec_p.bind(
            *operands, out_avals=tuple(out_avals), in_names=tuple(all_in_names),
            out_names=tuple(out_names), lowering_input_output_aliases=(),
            sim_require_finite=True, sim_require_nnan=True, nc=nc)
        return tuple(outs)

    devices = jax.devices()[:N_CORES]
    mesh = Mesh(np.asarray(devices), ("core",))
    n_outs = len(out_names)
    fn = jax.jit(
        shard_map(_body, mesh=mesh,
                  in_specs=(PartitionSpec("core"),) * (n_params + n_outs),
                  out_specs=(PartitionSpec("core"),) * n_outs,
                  check_rep=False),
        keep_unused=True)

    runner = {"fn": fn, "in_names": in_names, "out_names": out_names,
              "out_avals": out_avals, "zero_outs": zero_outs, "jax": jax}
    _CACHE["nc"] = nc
    _CACHE["runner"] = runner
    return runner


def build_chained(n_chain):
    """Jitted fn running the kernel n_chain times back-to-back (serialized via
    a tiny data dependency through bq) — for slope-based device timing."""
    r = _get_runner()
    import jax
    from jax.sharding import Mesh, PartitionSpec
    from jax.experimental.shard_map import shard_map
    from concourse.bass2jax import _bass_exec_p, partition_id_tensor

    nc = _CACHE["nc"]
    partition_name = nc.partition_id_tensor.name if nc.partition_id_tensor else None
    in_names = r["in_names"]
    out_names = r["out_names"]
    out_avals = r["out_avals"]
    n_params = len(in_names)
    all_in_names = list(in_names) + list(out_names)
    if partition_name is not None:
        all_in_names.append(partition_name)
    bq_idx = in_names.index("bq")
    yt_idx = out_names.index("yT")

    def _once(args):
        operands = list(args)
        if partition_name is not None:
            operands.append(partition_id_tensor())
        return _bass_exec_p.bind(
            *operands, out_avals=tuple(out_avals), in_names=tuple(all_in_names),
            out_names=tuple(out_names), lowering_input_output_aliases=(),
            sim_require_finite=True, sim_require_nnan=True, nc=nc)

    def _body(*args):
        args = list(args)
        outs = _once(args)
        for _ in range(n_chain - 1):
            # serialize: call i's output becomes call i+1's output buffer
            args[n_params + yt_idx] = outs[yt_idx]
            outs = _once(args)
        return tuple(outs)

    devices = jax.devices()[:N_CORES]
    mesh = Mesh(np.asarray(devices), ("core",))
    n_outs = len(out_names)
    return jax.jit(
        shard_map(_body, mesh=mesh,
                  in_specs=(PartitionSpec("core"),) * (n_params + n_outs),
                  out_specs=(PartitionSpec("core"),) * n_outs,
                  check_rep=False),
        keep_unused=True)


def _shard_inputs(query, key, value, Wq, bq, Wk, bk, Wv, bv, Wo, bo):
    """Build per-core input dict list."""
    q32 = np.asarray(query, dtype=np.float32)
    k32 = np.asarray(key, dtype=np.float32)
    v32 = np.asarray(value, dtype=np.float32)
    xT = [np.ascontiguousarray(a.transpose(0, 2, 1)) for a in (q32, k32, v32)]
    Wq, Wk, Wv, Wo = (np.asarray(a, np.float32) for a in (Wq, Wk, Wv, Wo))
    bqv, bkv, bvv = (np.asarray(a, np.float32).reshape(1, -1) for a in (bq, bk, bv))
    in_maps = []
    for c in range(N_CORES):
        b, g = divmod(c, HEADS_PER_CORE)
        j0 = g * JS
        in_maps.append({
            "xqT": xT[0][b], "xkT": xT[1][b], "xvT": xT[2][b],
            "wqT": np.ascontiguousarray(Wq[j0:j0 + JS].T),
            "wkT": np.ascontiguousarray(Wk[j0:j0 + JS].T),
            "wvT": np.ascontiguousarray(Wv[j0:j0 + JS].T),
            "woT": np.ascontiguousarray(Wo[:, j0:j0 + JS].T),
            "bq": bqv[:, j0:j0 + JS], "bk": bkv[:, j0:j0 + JS],
            "bv": bvv[:, j0:j0 + JS],
        })
    return in_maps


def kernel(query, key, value, Wq, bq, Wk, bk, Wv, bv, Wo, bo):
    r = _get_runner()
    jax = r["jax"]
    in_maps = _shard_inputs(query, key, value, Wq, bq, Wk, bk, Wv, bv, Wo, bo)
    concat_in = [np.concatenate([in_maps[c][nm] for c in range(N_CORES)], axis=0)
                 for nm in r["in_names"]]
    concat_zeros = [np.zeros((N_CORES * z.shape[0], *z.shape[1:]), z.dtype)
                    for z in r["zero_outs"]]
    outs = r["fn"](*[jax.device_put(a) for a in concat_in + concat_zeros])
    jax.block_until_ready(outs)
    i = r["out_names"].index("yT")
    yT_all = np.asarray(outs[i]).reshape(N_CORES, E, S)
    bo32 = np.asarray(bo, np.float32)
    out = np.empty((B, S, E), np.float32)
    for b in range(B):
        acc = yT_all[4 * b:4 * b + 4].sum(axis=0)  # [E, S]
        out[b] = acc.T + bo32
    return out
